# revision 22
# baseline (speedup 1.0000x reference)
"""Cross multi-head attention + residual + LayerNorm on 8 Trainium2 NeuronCores.

Reference (per batch b):
    q = x_q @ Wq.T + bq ; k = x_kv @ Wk.T + bk ; v = x_kv @ Wv.T + bv
    per head: ctx = softmax(q k^T / sqrt(64)) v
    out = concat(ctx) @ Wo.T + bo ;  y = LayerNorm(out + x_q) * gamma + beta

Sharding (8 cores): data parallel on batch (2 groups of 4 cores), tensor
parallel on heads (4 of 16 heads per core). Each core computes q/k/v
projections for its 4 heads over the full sequences, attention, and a
partial output projection (its heads' slice of Wo columns); a bf16
ReduceScatter within each 4-core group sums the partials per i-band and
hands each core 1/4 of the rows, on which it applies bias + residual +
LayerNorm locally.

Performance structure:
  - Activations are transposed and cast to bf16 on the HOST; the kernel
    streams x^T tiles from HBM (no PE transposes).
  - All matmuls bf16 with fp32 PSUM accumulation.
  - Attention for the first i-band/pair is fused into the kv-projection
    loop (per 512-key tile), hiding the projection phase under the
    ACT-bound exp stream.
  - exp() reads [128, 2, 512] PSUM score tiles (2 banks, double
    buffered); softmax skips max-subtraction (scores ~ N(0,1)) and folds
    the 1/8 scale into the exp. The denominator comes from an all-ones
    column appended to V, and its reciprocal is broadcast across
    partitions with a K=1 PE matmul (no DRAM round trip).
  - Context PSUM is evacuated to SBUF immediately after accumulation so
    the 2 accumulator banks recycle without waiting on normalization.
  - Per-band output projection + bf16 ReduceScatter overlap later bands'
    attention; LayerNorm runs per received band.

Self-contained: hardcodes shapes for B=2, L=2048, E=1024, H=16, Dh=64.
"""

from contextlib import ExitStack

import numpy as np
import ml_dtypes

import concourse.bass as bass
import concourse.mybir as mybir
import concourse.tile as tile
from concourse.bass_test_utils import run_kernel

F32 = mybir.dt.float32
BF16 = mybir.dt.bfloat16
NP_BF16 = ml_dtypes.bfloat16

B = 2
L = 2048          # query and kv sequence length
E = 1024          # embed
H_LOC = 4         # heads per core
DH = 64
EC = E // 128     # 8 e-chunks
JC = L // 128     # 16 key chunks of 128
IT = 512          # i-tile (moving free dim) for scores/ctx
N_IT = L // IT    # 4
GROUPS = [[0, 1, 2, 3], [4, 5, 6, 7]]
LN_EPS = 1e-5


def make_attention_kernel(iters=1):
    def _k(tc, outs, ins):
        return _attention_body(tc, outs, ins, iters)
    return _k


def _attention_body(tc: tile.TileContext, outs, ins, iters):
    nc = tc.nc
    (out,) = outs            # [4, 128, 1024] f32: four row-bands of the output
    (xqT, xkvT, wqT, wkT, wvT, woT, bqk, bv, bobc, gamma, beta, xqr) = ins

    rs_in = [nc.dram_tensor(f"rs_in{k}", [IT, E], BF16) for k in range(N_IT)]
    rs_out = [nc.dram_tensor(f"rs_out{k}", [128, E], BF16) for k in range(N_IT)]

    ctx = ExitStack()
    singles = ctx.enter_context(tc.tile_pool(name="singles", bufs=1))
    big = ctx.enter_context(tc.tile_pool(name="big", bufs=1))
    xtp = ctx.enter_context(tc.tile_pool(name="xtp", bufs=1))
    kvp = ctx.enter_context(tc.tile_pool(name="kvp", bufs=1))
    ex_pool = ctx.enter_context(tc.tile_pool(name="ex", bufs=4))
    small = ctx.enter_context(tc.tile_pool(name="small", bufs=2))
    cep = ctx.enter_context(tc.tile_pool(name="cep", bufs=4))
    evac = ctx.enter_context(tc.tile_pool(name="evac", bufs=2))
    lnp = ctx.enter_context(tc.tile_pool(name="lnp", bufs=2))
    # PSUM budget (8 banks): scores 2 bufs x 2 banks + ctx 2 x 1 + proj/bc 2 x 1
    ps_proj = ctx.enter_context(tc.tile_pool(name="ps_proj", bufs=2, space="PSUM"))
    ps_sc = ctx.enter_context(tc.tile_pool(name="ps_sc", bufs=2, space="PSUM"))
    ps_ctx = ctx.enter_context(tc.tile_pool(name="ps_ctx", bufs=2, space="PSUM"))

    # ---- weights & constants -------------------------------------------------
    w_sb = {}
    for name, src, shape in (
        ("wk", wkT, [128, EC, 256]),
        ("wv", wvT, [128, EC, 256]),
        ("wq", wqT, [128, EC, 256]),
        ("wo", woT, [128, 2, E]),
    ):
        wt = singles.tile(shape, BF16, name=f"{name}_sb")
        nc.sync.dma_start(out=wt[:], in_=src.rearrange("(c p) n -> p c n", p=128))
        w_sb[name] = wt

    bqk_sb = singles.tile([128, 4], F32, name="bqk_sb")
    nc.sync.dma_start(out=bqk_sb[:], in_=bqk[:])
    bv_bc = singles.tile([128, 256], F32, name="bv_bc")
    nc.gpsimd.dma_start(out=bv_bc[:], in_=bv[None, :].to_broadcast([128, 256]))
    bo_bc = singles.tile([128, E], F32, name="bo_bc")
    nc.gpsimd.dma_start(out=bo_bc[:], in_=bobc[None, :].to_broadcast([128, E]))
    gamma_bc = singles.tile([128, E], F32, name="gamma_bc")
    nc.gpsimd.dma_start(out=gamma_bc[:], in_=gamma[None, :].to_broadcast([128, E]))
    beta_bc = singles.tile([128, E], F32, name="beta_bc")
    nc.gpsimd.dma_start(out=beta_bc[:], in_=beta[None, :].to_broadcast([128, E]))
    eps_sb = singles.tile([128, 1], F32, name="eps_sb")
    nc.vector.memset(eps_sb[:], LN_EPS)
    ones_t = singles.tile([128, 64], F32, name="ones_t")
    nc.vector.memset(ones_t[:], 1.0)

    # v tiles per jt: [128 part (j%128), 4 jc, 4*65]; col 64 of each head
    # block is the all-ones softmax-denominator column.
    v_t = []
    for jt in range(N_IT):
        vt = big.tile([128, 4, H_LOC * 65], BF16, name=f"v_{jt}")
        vv = vt.rearrange("p a (h d) -> p (a h) d", d=65)
        nc.vector.memset(vv[:, :, 64:65], 1.0)
        v_t.append(vt)

    # kT tiles per (pair, jt): [128 d-of-pair, 512 j]
    kT = {(pair, jt): big.tile([128, IT], BF16, name=f"kT_{pair}_{jt}")
          for pair in range(2) for jt in range(N_IT)}
    # qT tiles per it: [128 d-of-pair, pair, 512 i]
    qT = [big.tile([128, 2, IT], BF16, name=f"qT_{it}") for it in range(N_IT)]
    ctxT_sb = big.tile([128, 2, L], BF16, name="ctxT_sb")  # [hd%128, hd//128, i]

    def _proj_q(it, xt):
        for pair in range(2):
            pq = ps_proj.tile([128, IT], F32, name=f"pq_{it}_{pair}", tag="pp")
            for ec in range(EC):
                nc.tensor.matmul(
                    pq[:],
                    w_sb["wq"][:, ec, pair * 128 : (pair + 1) * 128],
                    xt[:, ec, :],
                    start=(ec == 0),
                    stop=(ec == EC - 1),
                )
            nc.vector.tensor_scalar(
                out=qT[it][:, pair, :],
                in0=pq[:],
                scalar1=bqk_sb[:, pair : pair + 1],
                scalar2=None,
                op0=mybir.AluOpType.add,
            )

    def _proj_k(jt, pair, xt):
        pk = ps_proj.tile([128, IT], F32, name=f"pk_{jt}_{pair}", tag="pp")
        for ec in range(EC):
            nc.tensor.matmul(
                pk[:],
                w_sb["wk"][:, ec, pair * 128 : (pair + 1) * 128],
                xt[:, ec, :],
                start=(ec == 0),
                stop=(ec == EC - 1),
            )
        nc.vector.tensor_scalar(
            out=kT[(pair, jt)][:],
            in0=pk[:],
            scalar1=bqk_sb[:, 2 + pair : 3 + pair],
            scalar2=None,
            op0=mybir.AluOpType.add,
        )

    def _proj_v(jt, xt):
        for jj in range(4):
            pv = ps_proj.tile([128, 256], F32, name=f"pv_{jt}_{jj}", tag="pp")
            for ec in range(EC):
                nc.tensor.matmul(
                    pv[:],
                    xt[:, ec, jj * 128 : (jj + 1) * 128],
                    w_sb["wv"][:, ec, :],
                    start=(ec == 0),
                    stop=(ec == EC - 1),
                )
            nc.vector.tensor_tensor(
                out=v_t[jt].rearrange("p a (h d) -> p a h d", d=65)[
                    :, jj, :, 0:64
                ],
                in0=pv.rearrange("p (h d) -> p h d", d=64),
                in1=bv_bc.rearrange("p (h d) -> p h d", d=64),
                op=mybir.AluOpType.add,
            )

    def _attn_jc(it, pair, jc, pc_a, pc_b):
        ha, hb = 2 * pair, 2 * pair + 1
        jt, jj = divmod(jc, 4)
        s_ps = ps_sc.tile([128, 2, IT], F32, name=f"sps_{it}_{pair}_{jc}",
                          tag="sc")
        nc.tensor.matmul(
            s_ps[:, 0, :],
            kT[(pair, jt)][0:64, jj * 128 : (jj + 1) * 128],
            qT[it][0:64, pair, :],
            start=True, stop=True,
            tile_position=(0, 0),
        )
        nc.tensor.matmul(
            s_ps[:, 1, :],
            kT[(pair, jt)][64:128, jj * 128 : (jj + 1) * 128],
            qT[it][64:128, pair, :],
            start=True, stop=True,
            tile_position=(64, 0),
        )
        ex = ex_pool.tile([128, 2, IT], BF16, name=f"ex_{it}_{pair}_{jc}",
                          tag="ex")
        nc.scalar.activation(
            out=ex[:], in_=s_ps[:],
            func=mybir.ActivationFunctionType.Exp,
            scale=0.125,
        )
        nc.tensor.matmul(
            pc_a[0:65, :],
            v_t[jt][:, jj, ha * 65 : (ha + 1) * 65],
            ex[:, 0, :],
            start=(jc == 0), stop=(jc == JC - 1),
        )
        nc.tensor.matmul(
            pc_b[0:65, :],
            v_t[jt][:, jj, hb * 65 : (hb + 1) * 65],
            ex[:, 1, :],
            start=(jc == 0), stop=(jc == JC - 1),
        )

    def _normalize(it, pair, pc_a, pc_b):
        """Evacuate [ctx; denom] PSUM, reciprocal + PE-broadcast, scale."""
        isl = slice(it * IT, (it + 1) * IT)
        ha, hb = 2 * pair, 2 * pair + 1
        for head, pc in ((ha, pc_a), (hb, pc_b)):
            ce = cep.tile([128, IT], F32, name=f"ce_{it}_{head}", tag="ce")
            nc.vector.tensor_copy(ce[0:65, :], pc[0:65, :])
            rc = small.tile([128, IT], F32, name=f"rc_{it}_{head}", tag="rc")
            nc.vector.reciprocal(rc[64:65, :], ce[64:65, :])
            bc_ps = ps_proj.tile([128, IT], F32, name=f"bc_{it}_{head}",
                                 tag="pp")
            nc.tensor.matmul(
                bc_ps[0:64, :],
                ones_t[64:65, :],
                rc[64:65, :],
                start=True, stop=True,
                tile_position=(64, 0),
            )
            chunk = head // 2
            if head % 2 == 0:
                nc.vector.tensor_tensor(
                    out=ctxT_sb[0:64, chunk, isl],
                    in0=ce[0:64, :], in1=bc_ps[0:64, :],
                    op=mybir.AluOpType.mult,
                )
            else:
                sc = small.tile([64, IT], BF16, name=f"sc_{it}_{head}",
                                tag="scm")
                nc.vector.tensor_tensor(
                    out=sc[:], in0=ce[0:64, :], in1=bc_ps[0:64, :],
                    op=mybir.AluOpType.mult,
                )
                nc.gpsimd.dma_start(
                    out=ctxT_sb[64:128, chunk, isl], in_=sc[:],
                )

    def _attn_pair(it, pair):
        pc_a = ps_ctx.tile([128, IT], F32, name=f"pca_{it}_{pair}", tag="pc")
        pc_b = ps_ctx.tile([128, IT], F32, name=f"pcb_{it}_{pair}", tag="pc")
        for jc in range(JC):
            _attn_jc(it, pair, jc, pc_a, pc_b)
        _normalize(it, pair, pc_a, pc_b)

    def _outproj_band(it, do_tail):
        for i2 in range(4):
            ic = it * 4 + i2
            ot = evac.tile([128, E], BF16, name=f"ot_{ic}", tag="ot")
            for et in range(2):
                po = ps_proj.tile([128, IT], F32, name=f"po_{ic}_{et}",
                                  tag="pp")
                for hc in range(2):
                    nc.tensor.matmul(
                        po[:],
                        ctxT_sb[:, hc, ic * 128 : (ic + 1) * 128],
                        w_sb["wo"][:, hc, et * 512 : (et + 1) * 512],
                        start=(hc == 0),
                        stop=(hc == 1),
                    )
                nc.vector.tensor_copy(ot[:, et * 512 : (et + 1) * 512], po[:])
            nc.sync.dma_start(
                out=rs_in[it][i2 * 128 : (i2 + 1) * 128, :], in_=ot[:]
            )
        if do_tail:
            nc.gpsimd.collective_compute(
                "ReduceScatter",
                mybir.AluOpType.add,
                replica_groups=GROUPS,
                ins=[rs_in[it].ap()],
                outs=[rs_out[it].ap()],
            )

    def _ln_tail(do_tail):
        for band in (range(N_IT) if do_tail else []):
            xb = lnp.tile([128, E], BF16, name=f"lb_{band}", tag="lb")
            nc.sync.dma_start(out=xb[:], in_=rs_out[band].ap())
            xr = lnp.tile([128, E], F32, name=f"lr_{band}", tag="lr")
            nc.sync.dma_start(out=xr[:], in_=xqr[band])
            xt = lnp.tile([128, E], F32, name=f"lx_{band}", tag="lx")
            nc.vector.tensor_tensor(out=xt[:], in0=xb[:], in1=xr[:],
                                    op=mybir.AluOpType.add)
            nc.vector.tensor_tensor(out=xt[:], in0=xt[:], in1=bo_bc[:],
                                    op=mybir.AluOpType.add)
            stats = small.tile([128, 2, 6], F32, name=f"st_{band}", tag="st")
            for h in range(2):
                nc.vector.bn_stats(out=stats[:, h, :],
                                   in_=xt[:, h * 512 : (h + 1) * 512])
            mv = small.tile([128, 2], F32, name=f"mv_{band}", tag="mv")
            nc.vector.bn_aggr(out=mv[:], in_=stats.rearrange("p a b -> p (a b)"))
            rstd = small.tile([128, 1], F32, name=f"rstd_{band}", tag="rstd")
            nc.scalar.activation(
                out=rstd[:], in_=mv[:, 1:2],
                func=mybir.ActivationFunctionType.Sqrt,
                bias=eps_sb[:],
            )
            nc.vector.reciprocal(rstd[:], rstd[:])
            nc.vector.tensor_scalar(
                out=xt[:], in0=xt[:],
                scalar1=mv[:, 0:1], scalar2=rstd[:],
                op0=mybir.AluOpType.subtract,
                op1=mybir.AluOpType.mult,
            )
            nc.vector.tensor_tensor(out=xt[:], in0=xt[:], in1=gamma_bc[:],
                                    op=mybir.AluOpType.mult)
            nc.vector.tensor_tensor(out=xt[:], in0=xt[:], in1=beta_bc[:],
                                    op=mybir.AluOpType.add)
            nc.sync.dma_start(out=out[band], in_=xt[:])

    def body(do_tail=True):
        # input DMAs in priority order: q tile 0, then the four kv tiles,
        # then the remaining q tiles
        xq_tiles = []
        for it in range(N_IT):
            xt = xtp.tile([128, EC, IT], BF16, name=f"xqT_{it}", tag=f"xq{it}")
            xq_tiles.append(xt)
        nc.sync.dma_start(
            out=xq_tiles[0][:],
            in_=xqT.rearrange("(c p) n -> p c n", p=128)[:, :, 0:IT],
        )
        kv_tiles = []
        for jt in range(N_IT):
            xt = kvp.tile([128, EC, IT], BF16, name=f"xkvT_{jt}", tag=f"xkv{jt}")
            nc.sync.dma_start(
                out=xt[:],
                in_=xkvT.rearrange("(c p) n -> p c n", p=128)[
                    :, :, jt * IT : (jt + 1) * IT
                ],
            )
            kv_tiles.append(xt)
        for it in range(1, N_IT):
            nc.sync.dma_start(
                out=xq_tiles[it][:],
                in_=xqT.rearrange("(c p) n -> p c n", p=128)[
                    :, :, it * IT : (it + 1) * IT
                ],
            )

        # q projection for band 0, then kv projections fused with
        # attention for (it=0, pair=0)
        _proj_q(0, xq_tiles[0])
        pc00_a = ps_ctx.tile([128, IT], F32, name="pca_0_0", tag="pc")
        pc00_b = ps_ctx.tile([128, IT], F32, name="pcb_0_0", tag="pc")
        for jt in range(N_IT):
            _proj_k(jt, 0, kv_tiles[jt])
            _proj_v(jt, kv_tiles[jt])
            _proj_k(jt, 1, kv_tiles[jt])
            for jj in range(4):
                _attn_jc(0, 0, jt * 4 + jj, pc00_a, pc00_b)
        _normalize(0, 0, pc00_a, pc00_b)
        _attn_pair(0, 1)
        _outproj_band(0, do_tail)
        for it in range(1, N_IT):
            _proj_q(it, xq_tiles[it])
            _attn_pair(it, 0)
            _attn_pair(it, 1)
            _outproj_band(it, do_tail)
        _ln_tail(do_tail)

    if iters == 1:
        body()
    else:
        with tc.For_i(0, iters):
            body(do_tail=False)
        body()

    ctx.close()


def _prepare_inputs(query_seq, key_value_seq, Wq, bq, Wk, bk, Wv, bv, Wo, bo,
                    ln_gamma, ln_beta):
    """Build the 8 per-core input tuples (host-side transpose + bf16 cast)."""
    ins = []
    xT = [np.ascontiguousarray(query_seq[b].T).astype(NP_BF16) for b in range(2)]
    kvT = [np.ascontiguousarray(key_value_seq[b].T).astype(NP_BF16)
           for b in range(2)]
    for c in range(8):
        b, r = divmod(c, 4)
        hs = slice(256 * r, 256 * (r + 1))
        wqT = np.ascontiguousarray(Wq[hs, :].T).astype(NP_BF16)
        wkT = np.ascontiguousarray(Wk[hs, :].T).astype(NP_BF16)
        wvT = np.ascontiguousarray(Wv[hs, :].T).astype(NP_BF16)
        woT = np.ascontiguousarray(Wo[:, hs].T).astype(NP_BF16)
        bqk = np.stack(
            [bq[hs][:128], bq[hs][128:], bk[hs][:128], bk[hs][128:]], axis=1
        ).astype(np.float32)
        bvs = np.ascontiguousarray(bv[hs])
        # residual rows: band k covers batch rows [512k + 128r, 512k + 128(r+1))
        xqr = np.stack(
            [query_seq[b, 512 * k + 128 * r : 512 * k + 128 * (r + 1)]
             for k in range(4)]
        )
        ins.append((xT[b], kvT[b], wqT, wkT, wvT, woT, bqk, bvs,
                    np.ascontiguousarray(bo), np.ascontiguousarray(ln_gamma),
                    np.ascontiguousarray(ln_beta), xqr))
    return ins


def kernel(**inputs) -> np.ndarray:
    query_seq = np.asarray(inputs["query_seq"], dtype=np.float32)
    key_value_seq = np.asarray(inputs["key_value_seq"], dtype=np.float32)
    args = {
        k: np.asarray(inputs[k], dtype=np.float32)
        for k in ("Wq", "bq", "Wk", "bk", "Wv", "bv", "Wo", "bo",
                  "ln_gamma", "ln_beta")
    }
    ins = _prepare_inputs(query_seq, key_value_seq, **args)
    out_like = [(np.zeros((4, 128, E), np.float32),) for _ in range(8)]
    res = run_kernel(
        make_attention_kernel(1),
        None,
        ins,
        bass_type=tile.TileContext,
        num_cores=8,
        check_with_sim=False,
        check_with_hw=True,
        trace_hw=False,
        output_like=out_like,
    )
    out = np.empty((B, L, E), np.float32)
    for c in range(8):
        bnd = res.results[c]["0_dram"]  # [4, 128, 1024]
        b, r = divmod(c, 4)
        for k in range(4):
            out[b, 512 * k + 128 * r : 512 * k + 128 * (r + 1), :] = bnd[k]
    return out


# revision 30
# speedup vs baseline: 1.1436x; 1.1436x over previous
"""Cross multi-head attention + residual + LayerNorm on 8 Trainium2 NeuronCores.

Reference (per batch b):
    q = x_q @ Wq.T + bq ; k = x_kv @ Wk.T + bk ; v = x_kv @ Wv.T + bv
    per head: ctx = softmax(q k^T / sqrt(64)) v
    out = concat(ctx) @ Wo.T + bo ;  y = LayerNorm(out + x_q) * gamma + beta

Sharding (8 cores): data parallel on batch (2 groups of 4 cores), tensor
parallel on heads (4 of 16 heads per core). Each core computes q/k/v
projections for its 4 heads over the full sequences, attention, and a
partial output projection (its heads' slice of Wo columns); a bf16
ReduceScatter within each 4-core group sums the partials per i-band and
hands each core 1/4 of the rows, on which it applies bias + residual +
LayerNorm locally.

Performance structure:
  - Activations are transposed and cast to bf16 on the HOST; the kernel
    streams x^T tiles from HBM (no PE transposes).
  - All matmuls bf16 with fp32 PSUM accumulation.
  - Attention for the first i-band/pair is fused into the kv-projection
    loop (per 512-key tile), hiding the projection phase under the
    ACT-bound exp stream.
  - exp() reads [128, 2, 512] PSUM score tiles (2 banks, double
    buffered); softmax skips max-subtraction (scores ~ N(0,1)) and folds
    the 1/8 scale into the exp. The denominator comes from an all-ones
    column appended to V, and its reciprocal is broadcast across
    partitions with a K=1 PE matmul (no DRAM round trip).
  - Context PSUM is evacuated to SBUF immediately after accumulation so
    the 2 accumulator banks recycle without waiting on normalization.
  - Per-band output projection + bf16 ReduceScatter overlap later bands'
    attention; LayerNorm runs per received band.

Self-contained: hardcodes shapes for B=2, L=2048, E=1024, H=16, Dh=64.
"""

from contextlib import ExitStack

import numpy as np
import ml_dtypes

import concourse.bass as bass
import concourse.mybir as mybir
import concourse.tile as tile
from concourse.bass_test_utils import run_kernel

F32 = mybir.dt.float32
BF16 = mybir.dt.bfloat16
NP_BF16 = ml_dtypes.bfloat16

B = 2
L = 2048          # query and kv sequence length
E = 1024          # embed
H_LOC = 4         # heads per core
DH = 64
EC = E // 128     # 8 e-chunks
JC = L // 128     # 16 key chunks of 128
IT = 512          # i-tile (moving free dim) for scores/ctx
N_IT = L // IT    # 4
GROUPS = [[0, 1, 2, 3], [4, 5, 6, 7]]
LN_EPS = 1e-5


def make_attention_kernel(iters=1):
    def _k(tc, outs, ins):
        return _attention_body(tc, outs, ins, iters)
    return _k


def _attention_body(tc: tile.TileContext, outs, ins, iters):
    nc = tc.nc
    (out,) = outs            # [4, 128, 1024] f32: four row-bands of the output
    (xqT, xkvT, wqT, wkT, wvT, woT, bqk, bv, bobc, gamma, beta, xqr) = ins

    rs_in = [nc.dram_tensor(f"rs_in{k}", [IT, E], BF16) for k in range(N_IT)]
    rs_out = [nc.dram_tensor(f"rs_out{k}", [128, E], BF16) for k in range(N_IT)]

    ctx = ExitStack()
    singles = ctx.enter_context(tc.tile_pool(name="singles", bufs=1))
    big = ctx.enter_context(tc.tile_pool(name="big", bufs=1))
    xtp = ctx.enter_context(tc.tile_pool(name="xtp", bufs=1))
    kvp = ctx.enter_context(tc.tile_pool(name="kvp", bufs=1))
    ex_pool = ctx.enter_context(tc.tile_pool(name="ex", bufs=4))
    small = ctx.enter_context(tc.tile_pool(name="small", bufs=2))
    cep = ctx.enter_context(tc.tile_pool(name="cep", bufs=4))
    evac = ctx.enter_context(tc.tile_pool(name="evac", bufs=2))
    lnp = ctx.enter_context(tc.tile_pool(name="lnp", bufs=2))
    # PSUM budget (8 banks): scores 2 bufs x 2 banks + ctx 2 x 1 + proj/bc 2 x 1
    ps_proj = ctx.enter_context(tc.tile_pool(name="ps_proj", bufs=2, space="PSUM"))
    ps_sc = ctx.enter_context(tc.tile_pool(name="ps_sc", bufs=2, space="PSUM"))
    ps_ctx = ctx.enter_context(tc.tile_pool(name="ps_ctx", bufs=2, space="PSUM"))

    # ---- weights & constants -------------------------------------------------
    w_sb = {}
    for name, src, shape in (
        ("wk", wkT, [128, EC, 256]),
        ("wv", wvT, [128, EC, 256]),
        ("wq", wqT, [128, EC, 256]),
        ("wo", woT, [128, 2, E]),
    ):
        wt = singles.tile(shape, BF16, name=f"{name}_sb")
        nc.sync.dma_start(out=wt[:], in_=src.rearrange("(c p) n -> p c n", p=128))
        w_sb[name] = wt

    bqk_sb = singles.tile([128, 4], F32, name="bqk_sb")
    nc.sync.dma_start(out=bqk_sb[:], in_=bqk[:])
    bv_bc = singles.tile([128, 256], F32, name="bv_bc")
    nc.gpsimd.dma_start(out=bv_bc[:], in_=bv[None, :].to_broadcast([128, 256]))
    bo_bc = singles.tile([128, E], F32, name="bo_bc")
    nc.gpsimd.dma_start(out=bo_bc[:], in_=bobc[None, :].to_broadcast([128, E]))
    gamma_bc = singles.tile([128, E], F32, name="gamma_bc")
    nc.gpsimd.dma_start(out=gamma_bc[:], in_=gamma[None, :].to_broadcast([128, E]))
    beta_bc = singles.tile([128, E], F32, name="beta_bc")
    nc.gpsimd.dma_start(out=beta_bc[:], in_=beta[None, :].to_broadcast([128, E]))
    eps_sb = singles.tile([128, 1], F32, name="eps_sb")
    nc.vector.memset(eps_sb[:], LN_EPS)
    ones_t = singles.tile([128, 64], BF16, name="ones_t")
    nc.vector.memset(ones_t[:], 1.0)

    # v tiles per jt: [128 part (j%128), 4 jc, 4*65]; col 64 of each head
    # block is the all-ones softmax-denominator column.
    v_t = []
    for jt in range(N_IT):
        vt = big.tile([128, 4, H_LOC * 65], BF16, name=f"v_{jt}")
        vv = vt.rearrange("p a (h d) -> p (a h) d", d=65)
        nc.vector.memset(vv[:, :, 64:65], 1.0)
        v_t.append(vt)

    # kT tiles per (pair, jt): [128 d-of-pair, 512 j]
    kT = {(pair, jt): big.tile([128, IT], BF16, name=f"kT_{pair}_{jt}")
          for pair in range(2) for jt in range(N_IT)}
    # qT tiles per (it, pair): [128 d-of-pair, 512 i]
    qT = {(it, pair): big.tile([128, IT], BF16, name=f"qT_{it}_{pair}")
          for it in range(N_IT) for pair in range(2)}
    ctxT_sb = big.tile([128, 2, L], BF16, name="ctxT_sb")  # [hd%128, hd//128, i]

    def _proj_q(it, xt, pairs=(0, 1)):
        for pair in pairs:
            pq = ps_proj.tile([128, IT], F32, name=f"pq_{it}_{pair}", tag="pp")
            for ec in range(EC):
                nc.tensor.matmul(
                    pq[:],
                    w_sb["wq"][:, ec, pair * 128 : (pair + 1) * 128],
                    xt[:, ec, :],
                    start=(ec == 0),
                    stop=(ec == EC - 1),
                )
            nc.vector.tensor_scalar(
                out=qT[(it, pair)][:],
                in0=pq[:],
                scalar1=bqk_sb[:, pair : pair + 1],
                scalar2=None,
                op0=mybir.AluOpType.add,
            )

    def _proj_k(jt, pair, xt):
        pk = ps_proj.tile([128, IT], F32, name=f"pk_{jt}_{pair}", tag="pp")
        for ec in range(EC):
            nc.tensor.matmul(
                pk[:],
                w_sb["wk"][:, ec, pair * 128 : (pair + 1) * 128],
                xt[:, ec, :],
                start=(ec == 0),
                stop=(ec == EC - 1),
            )
        nc.vector.tensor_scalar(
            out=kT[(pair, jt)][:],
            in0=pk[:],
            scalar1=bqk_sb[:, 2 + pair : 3 + pair],
            scalar2=None,
            op0=mybir.AluOpType.add,
        )

    def _proj_v(jt, xt):
        for jj in range(4):
            pv = ps_proj.tile([128, 256], F32, name=f"pv_{jt}_{jj}", tag="pp")
            for ec in range(EC):
                nc.tensor.matmul(
                    pv[:],
                    xt[:, ec, jj * 128 : (jj + 1) * 128],
                    w_sb["wv"][:, ec, :],
                    start=(ec == 0),
                    stop=(ec == EC - 1),
                )
            nc.vector.tensor_tensor(
                out=v_t[jt].rearrange("p a (h d) -> p a h d", d=65)[
                    :, jj, :, 0:64
                ],
                in0=pv.rearrange("p (h d) -> p h d", d=64),
                in1=bv_bc.rearrange("p (h d) -> p h d", d=64),
                op=mybir.AluOpType.add,
            )

    def _attn_jc(it, pair, jc, pc_a, pc_b):
        ha, hb = 2 * pair, 2 * pair + 1
        jt, jj = divmod(jc, 4)
        s_ps = ps_sc.tile([128, 2, IT], F32, name=f"sps_{it}_{pair}_{jc}",
                          tag="sc")
        nc.tensor.matmul(
            s_ps[:, 0, :],
            kT[(pair, jt)][0:64, jj * 128 : (jj + 1) * 128],
            qT[(it, pair)][0:64, :],
            start=True, stop=True,
            tile_position=(0, 0),
        )
        nc.tensor.matmul(
            s_ps[:, 1, :],
            kT[(pair, jt)][64:128, jj * 128 : (jj + 1) * 128],
            qT[(it, pair)][64:128, :],
            start=True, stop=True,
            tile_position=(64, 0),
        )
        ex = ex_pool.tile([128, 2, IT], BF16, name=f"ex_{it}_{pair}_{jc}",
                          tag="ex")
        nc.scalar.activation(
            out=ex[:], in_=s_ps[:],
            func=mybir.ActivationFunctionType.Exp,
            scale=0.125,
        )
        nc.tensor.matmul(
            pc_a[0:65, :],
            v_t[jt][:, jj, ha * 65 : (ha + 1) * 65],
            ex[:, 0, :],
            start=(jc == 0), stop=(jc == JC - 1),
        )
        nc.tensor.matmul(
            pc_b[0:65, :],
            v_t[jt][:, jj, hb * 65 : (hb + 1) * 65],
            ex[:, 1, :],
            start=(jc == 0), stop=(jc == JC - 1),
        )

    def _normalize(it, pair, pc_a, pc_b):
        """Evacuate [ctx; denom] PSUM, reciprocal + PE-broadcast, scale."""
        isl = slice(it * IT, (it + 1) * IT)
        ha, hb = 2 * pair, 2 * pair + 1
        for head, pc in ((ha, pc_a), (hb, pc_b)):
            ce = cep.tile([128, IT], F32, name=f"ce_{it}_{head}", tag="ce")
            nc.vector.tensor_copy(ce[0:65, :], pc[0:65, :])
            rc = small.tile([128, IT], BF16, name=f"rc_{it}_{head}", tag="rc")
            with nc.allow_low_precision(
                reason="bf16 softmax-denominator reciprocal feeds the PE "
                       "broadcast matmul; 0.4% relative error is ~1e-4 of "
                       "the final output"
            ):
                nc.vector.reciprocal(rc[64:65, :], ce[64:65, :])
            bc_ps = ps_proj.tile([128, IT], F32, name=f"bc_{it}_{head}",
                                 tag="pp")
            nc.tensor.matmul(
                bc_ps[0:64, :],
                ones_t[64:65, :],
                rc[64:65, :],
                start=True, stop=True,
                tile_position=(64, 0),
            )
            chunk = head // 2
            if head % 2 == 0:
                nc.vector.tensor_tensor(
                    out=ctxT_sb[0:64, chunk, isl],
                    in0=ce[0:64, :], in1=bc_ps[0:64, :],
                    op=mybir.AluOpType.mult,
                )
            else:
                sc = small.tile([64, IT], BF16, name=f"sc_{it}_{head}",
                                tag="scm")
                nc.vector.tensor_tensor(
                    out=sc[:], in0=ce[0:64, :], in1=bc_ps[0:64, :],
                    op=mybir.AluOpType.mult,
                )
                nc.gpsimd.dma_start(
                    out=ctxT_sb[64:128, chunk, isl], in_=sc[:],
                )

    def _attn_pair(it, pair):
        pc_a = ps_ctx.tile([128, IT], F32, name=f"pca_{it}_{pair}", tag="pc")
        pc_b = ps_ctx.tile([128, IT], F32, name=f"pcb_{it}_{pair}", tag="pc")
        for jc in range(JC):
            _attn_jc(it, pair, jc, pc_a, pc_b)
        _normalize(it, pair, pc_a, pc_b)

    def _outproj_band(it, do_tail, split_evac=False):
        for i2 in range(4):
            ic = it * 4 + i2
            ot = evac.tile([128, E], BF16, name=f"ot_{ic}", tag="ot")
            for et in range(2):
                po = ps_proj.tile([128, IT], F32, name=f"po_{ic}_{et}",
                                  tag="pp")
                for hc in range(2):
                    nc.tensor.matmul(
                        po[:],
                        ctxT_sb[:, hc, ic * 128 : (ic + 1) * 128],
                        w_sb["wo"][:, hc, et * 512 : (et + 1) * 512],
                        start=(hc == 0),
                        stop=(hc == 1),
                    )
                dst = ot[:, et * 512 : (et + 1) * 512]
                if split_evac and et == 0:
                    nc.scalar.copy(dst, po[:])
                else:
                    nc.vector.tensor_copy(dst, po[:])
            nc.sync.dma_start(
                out=rs_in[it][i2 * 128 : (i2 + 1) * 128, :], in_=ot[:]
            )
        if do_tail:
            nc.gpsimd.collective_compute(
                "ReduceScatter",
                mybir.AluOpType.add,
                replica_groups=GROUPS,
                ins=[rs_in[it].ap()],
                outs=[rs_out[it].ap()],
            )

    def _ln_tail(do_tail):
        for band in (range(N_IT) if do_tail else []):
            xb = lnp.tile([128, E], BF16, name=f"lb_{band}", tag="lb")
            nc.sync.dma_start(out=xb[:], in_=rs_out[band].ap())
            xr = lnp.tile([128, E], F32, name=f"lr_{band}", tag="lr")
            nc.sync.dma_start(out=xr[:], in_=xqr[band])
            xt = lnp.tile([128, E], F32, name=f"lx_{band}", tag="lx")
            nc.vector.tensor_tensor(out=xt[:], in0=xb[:], in1=xr[:],
                                    op=mybir.AluOpType.add)
            nc.vector.tensor_tensor(out=xt[:], in0=xt[:], in1=bo_bc[:],
                                    op=mybir.AluOpType.add)
            stats = small.tile([128, 2, 6], F32, name=f"st_{band}", tag="st")
            for h in range(2):
                nc.vector.bn_stats(out=stats[:, h, :],
                                   in_=xt[:, h * 512 : (h + 1) * 512])
            mv = small.tile([128, 2], F32, name=f"mv_{band}", tag="mv")
            nc.vector.bn_aggr(out=mv[:], in_=stats.rearrange("p a b -> p (a b)"))
            rstd = small.tile([128, 1], F32, name=f"rstd_{band}", tag="rstd")
            nc.scalar.activation(
                out=rstd[:], in_=mv[:, 1:2],
                func=mybir.ActivationFunctionType.Sqrt,
                bias=eps_sb[:],
            )
            nc.vector.reciprocal(rstd[:], rstd[:])
            nc.vector.tensor_scalar(
                out=xt[:], in0=xt[:],
                scalar1=mv[:, 0:1], scalar2=rstd[:],
                op0=mybir.AluOpType.subtract,
                op1=mybir.AluOpType.mult,
            )
            nc.vector.tensor_tensor(out=xt[:], in0=xt[:], in1=gamma_bc[:],
                                    op=mybir.AluOpType.mult)
            nc.vector.tensor_tensor(out=xt[:], in0=xt[:], in1=beta_bc[:],
                                    op=mybir.AluOpType.add)
            nc.sync.dma_start(out=out[band], in_=xt[:])

    def body(do_tail=True):
        # input DMAs in priority order: q tile 0, then the four kv tiles,
        # then the remaining q tiles
        xq_tiles = []
        for it in range(N_IT):
            xt = xtp.tile([128, EC, IT], BF16, name=f"xqT_{it}", tag=f"xq{it}")
            xq_tiles.append(xt)
        nc.sync.dma_start(
            out=xq_tiles[0][:],
            in_=xqT.rearrange("(c p) n -> p c n", p=128)[:, :, 0:IT],
        )
        kv_tiles = []
        for jt in range(N_IT):
            xt = kvp.tile([128, EC, IT], BF16, name=f"xkvT_{jt}", tag=f"xkv{jt}")
            nc.sync.dma_start(
                out=xt[:],
                in_=xkvT.rearrange("(c p) n -> p c n", p=128)[
                    :, :, jt * IT : (jt + 1) * IT
                ],
            )
            kv_tiles.append(xt)
        for it in range(1, N_IT):
            nc.sync.dma_start(
                out=xq_tiles[it][:],
                in_=xqT.rearrange("(c p) n -> p c n", p=128)[
                    :, :, it * IT : (it + 1) * IT
                ],
            )

        # q projection for (band 0, pair 0), then kv projections fused with
        # attention for (it=0, pair=0)
        _proj_q(0, xq_tiles[0], pairs=(0,))
        pc00_a = ps_ctx.tile([128, IT], F32, name="pca_0_0", tag="pc")
        pc00_b = ps_ctx.tile([128, IT], F32, name="pcb_0_0", tag="pc")
        for jt in range(N_IT):
            _proj_k(jt, 0, kv_tiles[jt])
            _proj_v(jt, kv_tiles[jt])
            for jj in range(4):
                _attn_jc(0, 0, jt * 4 + jj, pc00_a, pc00_b)
            _proj_k(jt, 1, kv_tiles[jt])
        _proj_q(0, xq_tiles[0], pairs=(1,))
        _normalize(0, 0, pc00_a, pc00_b)
        # steady state: q-proj for the next band runs between this band's
        # pairs; the previous band's output projection is placed after the
        # next attention pair so it fills PE slack under the ACT-bound
        # exp stream instead of blocking it
        _proj_q(1, xq_tiles[1])
        _attn_pair(0, 1)
        for it in range(1, N_IT):
            _attn_pair(it, 0)
            if it < N_IT - 1:
                _proj_q(it + 1, xq_tiles[it + 1])
            _outproj_band(it - 1, do_tail)
            _attn_pair(it, 1)
        _outproj_band(N_IT - 1, do_tail, split_evac=True)
        _ln_tail(do_tail)

    if iters == 1:
        body()
    else:
        with tc.For_i(0, iters):
            body(do_tail=False)
        body()

    ctx.close()


def _prepare_inputs(query_seq, key_value_seq, Wq, bq, Wk, bk, Wv, bv, Wo, bo,
                    ln_gamma, ln_beta):
    """Build the 8 per-core input tuples (host-side transpose + bf16 cast)."""
    ins = []
    xT = [np.ascontiguousarray(query_seq[b].T).astype(NP_BF16) for b in range(2)]
    kvT = [np.ascontiguousarray(key_value_seq[b].T).astype(NP_BF16)
           for b in range(2)]
    for c in range(8):
        b, r = divmod(c, 4)
        hs = slice(256 * r, 256 * (r + 1))
        wqT = np.ascontiguousarray(Wq[hs, :].T).astype(NP_BF16)
        wkT = np.ascontiguousarray(Wk[hs, :].T).astype(NP_BF16)
        wvT = np.ascontiguousarray(Wv[hs, :].T).astype(NP_BF16)
        woT = np.ascontiguousarray(Wo[:, hs].T).astype(NP_BF16)
        bqk = np.stack(
            [bq[hs][:128], bq[hs][128:], bk[hs][:128], bk[hs][128:]], axis=1
        ).astype(np.float32)
        bvs = np.ascontiguousarray(bv[hs])
        # residual rows: band k covers batch rows [512k + 128r, 512k + 128(r+1))
        xqr = np.stack(
            [query_seq[b, 512 * k + 128 * r : 512 * k + 128 * (r + 1)]
             for k in range(4)]
        )
        ins.append((xT[b], kvT[b], wqT, wkT, wvT, woT, bqk, bvs,
                    np.ascontiguousarray(bo), np.ascontiguousarray(ln_gamma),
                    np.ascontiguousarray(ln_beta), xqr))
    return ins


def kernel(**inputs) -> np.ndarray:
    query_seq = np.asarray(inputs["query_seq"], dtype=np.float32)
    key_value_seq = np.asarray(inputs["key_value_seq"], dtype=np.float32)
    args = {
        k: np.asarray(inputs[k], dtype=np.float32)
        for k in ("Wq", "bq", "Wk", "bk", "Wv", "bv", "Wo", "bo",
                  "ln_gamma", "ln_beta")
    }
    ins = _prepare_inputs(query_seq, key_value_seq, **args)
    out_like = [(np.zeros((4, 128, E), np.float32),) for _ in range(8)]
    res = run_kernel(
        make_attention_kernel(1),
        None,
        ins,
        bass_type=tile.TileContext,
        num_cores=8,
        check_with_sim=False,
        check_with_hw=True,
        trace_hw=False,
        output_like=out_like,
    )
    out = np.empty((B, L, E), np.float32)
    for c in range(8):
        bnd = res.results[c]["0_dram"]  # [4, 128, 1024]
        b, r = divmod(c, 4)
        for k in range(4):
            out[b, 512 * k + 128 * r : 512 * k + 128 * (r + 1), :] = bnd[k]
    return out


# revision 36
# speedup vs baseline: 1.2240x; 1.0703x over previous
"""Cross multi-head attention + residual + LayerNorm on 8 Trainium2 NeuronCores.

Reference (per batch b):
    q = x_q @ Wq.T + bq ; k = x_kv @ Wk.T + bk ; v = x_kv @ Wv.T + bv
    per head: ctx = softmax(q k^T / sqrt(64)) v
    out = concat(ctx) @ Wo.T + bo ;  y = LayerNorm(out + x_q) * gamma + beta

Sharding (8 cores): data parallel on batch (2 groups of 4 cores), tensor
parallel on heads (4 of 16 heads per core). Each core computes q/k/v
projections for its 4 heads over the full sequences, attention, and a
partial output projection (its heads' slice of Wo columns); a bf16
ReduceScatter within each 4-core group sums the partials per i-band and
hands each core 1/4 of the rows, on which it applies bias + residual +
LayerNorm locally.

Performance structure:
  - Activations are transposed and cast to bf16 on the HOST; the kernel
    streams x^T tiles from HBM (no PE transposes).
  - All matmuls bf16 with fp32 PSUM accumulation.
  - Attention for the first i-band/pair is fused into the kv-projection
    loop (per 512-key tile), hiding the projection phase under the
    ACT-bound exp stream.
  - exp() reads [128, 2, 512] PSUM score tiles (2 banks, double
    buffered); softmax skips max-subtraction (scores ~ N(0,1)) and folds
    the 1/8 scale into the exp. The denominator comes from an all-ones
    column appended to V, and its reciprocal is broadcast across
    partitions with a K=1 PE matmul (no DRAM round trip).
  - Context PSUM is evacuated to SBUF immediately after accumulation so
    the 2 accumulator banks recycle without waiting on normalization.
  - Per-band output projection + bf16 ReduceScatter overlap later bands'
    attention; LayerNorm runs per received band.

Self-contained: hardcodes shapes for B=2, L=2048, E=1024, H=16, Dh=64.
"""

from contextlib import ExitStack

import numpy as np
import ml_dtypes

import concourse.bass as bass
import concourse.mybir as mybir
import concourse.tile as tile
from concourse.bass_test_utils import run_kernel

F32 = mybir.dt.float32
BF16 = mybir.dt.bfloat16
NP_BF16 = ml_dtypes.bfloat16

B = 2
L = 2048          # query and kv sequence length
E = 1024          # embed
H_LOC = 4         # heads per core
DH = 64
EC = E // 128     # 8 e-chunks
JC = L // 128     # 16 key chunks of 128
IT = 512          # i-tile (moving free dim) for scores/ctx
N_IT = L // IT    # 4
GROUPS = [[0, 1, 2, 3], [4, 5, 6, 7]]
LN_EPS = 1e-5


def make_attention_kernel(iters=1):
    def _k(tc, outs, ins):
        return _attention_body(tc, outs, ins, iters)
    return _k


def _attention_body(tc: tile.TileContext, outs, ins, iters):
    nc = tc.nc
    (out,) = outs            # [4, 128, 1024] f32: four row-bands of the output
    (xqT, xkvT, wqT, wkT, wvT, woT, bqk, bv, bobc, gamma, beta, xqr) = ins

    rs_in = [nc.dram_tensor(f"rs_in{k}", [IT, E], BF16) for k in range(N_IT)]
    rs_out = [nc.dram_tensor(f"rs_out{k}", [128, E], BF16) for k in range(N_IT)]

    ctx = ExitStack()
    singles = ctx.enter_context(tc.tile_pool(name="singles", bufs=1))
    big = ctx.enter_context(tc.tile_pool(name="big", bufs=1))
    xtp = ctx.enter_context(tc.tile_pool(name="xtp", bufs=1))
    kvp = ctx.enter_context(tc.tile_pool(name="kvp", bufs=1))
    ex_pool = ctx.enter_context(tc.tile_pool(name="ex", bufs=4))
    small = ctx.enter_context(tc.tile_pool(name="small", bufs=2))
    cep = ctx.enter_context(tc.tile_pool(name="cep", bufs=4))
    evac = ctx.enter_context(tc.tile_pool(name="evac", bufs=2))
    lnp = ctx.enter_context(tc.tile_pool(name="lnp", bufs=2))
    # PSUM budget (8 banks): scores 2 bufs x 2 banks + ctx 2 x 1 + proj/bc 2 x 1
    ps_proj = ctx.enter_context(tc.tile_pool(name="ps_proj", bufs=2, space="PSUM"))
    ps_sc = ctx.enter_context(tc.tile_pool(name="ps_sc", bufs=2, space="PSUM"))
    ps_ctx = ctx.enter_context(tc.tile_pool(name="ps_ctx", bufs=2, space="PSUM"))

    # ---- weights & constants -------------------------------------------------
    w_sb = {}
    for name, src, shape in (
        ("wk", wkT, [128, EC, 256]),
        ("wv", wvT, [128, EC, 256]),
        ("wq", wqT, [128, EC, 256]),
        ("wo", woT, [128, 2, E]),
    ):
        wt = singles.tile(shape, BF16, name=f"{name}_sb")
        nc.sync.dma_start(out=wt[:], in_=src.rearrange("(c p) n -> p c n", p=128))
        w_sb[name] = wt

    bqk_sb = singles.tile([128, 4], F32, name="bqk_sb")
    nc.sync.dma_start(out=bqk_sb[:], in_=bqk[:])
    bv_bc = singles.tile([128, 256], F32, name="bv_bc")
    nc.gpsimd.dma_start(out=bv_bc[:], in_=bv[None, :].to_broadcast([128, 256]))
    bo_bc = singles.tile([128, E], F32, name="bo_bc")
    nc.gpsimd.dma_start(out=bo_bc[:], in_=bobc[None, :].to_broadcast([128, E]))
    gamma_bc = singles.tile([128, E], F32, name="gamma_bc")
    nc.gpsimd.dma_start(out=gamma_bc[:], in_=gamma[None, :].to_broadcast([128, E]))
    beta_bc = singles.tile([128, E], F32, name="beta_bc")
    nc.gpsimd.dma_start(out=beta_bc[:], in_=beta[None, :].to_broadcast([128, E]))
    eps_sb = singles.tile([128, 1], F32, name="eps_sb")
    nc.vector.memset(eps_sb[:], LN_EPS)
    ones_t = singles.tile([128, 64], BF16, name="ones_t")
    nc.vector.memset(ones_t[:], 1.0)

    # v tiles per jt: [128 part (j%128), 4 jc, 4*65]; col 64 of each head
    # block is the all-ones softmax-denominator column.
    v_t = []
    for jt in range(N_IT):
        vt = big.tile([128, 4, H_LOC * 65], BF16, name=f"v_{jt}")
        vv = vt.rearrange("p a (h d) -> p (a h) d", d=65)
        nc.vector.memset(vv[:, :, 64:65], 1.0)
        v_t.append(vt)

    # kT tiles per (pair, jt): [128 d-of-pair, 512 j]
    kT = {(pair, jt): big.tile([128, IT], BF16, name=f"kT_{pair}_{jt}")
          for pair in range(2) for jt in range(N_IT)}
    # qT tiles per (it, pair): [128 d-of-pair, 512 i]
    qT = {(it, pair): big.tile([128, IT], BF16, name=f"qT_{it}_{pair}")
          for it in range(N_IT) for pair in range(2)}
    ctxT_sb = big.tile([128, 2, L], BF16, name="ctxT_sb")  # [hd%128, hd//128, i]

    def _proj_q(it, xt, pairs=(0, 1)):
        for pair in pairs:
            pq = ps_proj.tile([128, IT], F32, name=f"pq_{it}_{pair}", tag="pp")
            for ec in range(EC):
                nc.tensor.matmul(
                    pq[:],
                    w_sb["wq"][:, ec, pair * 128 : (pair + 1) * 128],
                    xt[:, ec, :],
                    start=(ec == 0),
                    stop=(ec == EC - 1),
                )
            nc.vector.tensor_scalar(
                out=qT[(it, pair)][:],
                in0=pq[:],
                scalar1=bqk_sb[:, pair : pair + 1],
                scalar2=None,
                op0=mybir.AluOpType.add,
            )

    def _proj_k(jt, pair, xt):
        pk = ps_proj.tile([128, IT], F32, name=f"pk_{jt}_{pair}", tag="pp")
        for ec in range(EC):
            nc.tensor.matmul(
                pk[:],
                w_sb["wk"][:, ec, pair * 128 : (pair + 1) * 128],
                xt[:, ec, :],
                start=(ec == 0),
                stop=(ec == EC - 1),
            )
        nc.vector.tensor_scalar(
            out=kT[(pair, jt)][:],
            in0=pk[:],
            scalar1=bqk_sb[:, 2 + pair : 3 + pair],
            scalar2=None,
            op0=mybir.AluOpType.add,
        )

    def _proj_v(jt, xt, half):
        hsl = slice(half * 128, (half + 1) * 128)
        for jj in range(4):
            pv = ps_proj.tile([128, 256], F32, name=f"pv_{jt}_{jj}_{half}",
                              tag="pp")
            for ec in range(EC):
                nc.tensor.matmul(
                    pv[:, 0:128],
                    xt[:, ec, jj * 128 : (jj + 1) * 128],
                    w_sb["wv"][:, ec, hsl],
                    start=(ec == 0),
                    stop=(ec == EC - 1),
                )
            nc.vector.tensor_tensor(
                out=v_t[jt].rearrange("p a (h d) -> p a h d", d=65)[
                    :, jj, 2 * half : 2 * half + 2, 0:64
                ],
                in0=pv[:, 0:128].rearrange("p (h d) -> p h d", d=64),
                in1=bv_bc[:, hsl].rearrange("p (h d) -> p h d", d=64),
                op=mybir.AluOpType.add,
            )

    def _attn_jc(it, pair, jc, pc_a, pc_b):
        ha, hb = 2 * pair, 2 * pair + 1
        jt, jj = divmod(jc, 4)
        s_ps = ps_sc.tile([128, 2, IT], F32, name=f"sps_{it}_{pair}_{jc}",
                          tag="sc")
        nc.tensor.matmul(
            s_ps[:, 0, :],
            kT[(pair, jt)][0:64, jj * 128 : (jj + 1) * 128],
            qT[(it, pair)][0:64, :],
            start=True, stop=True,
            tile_position=(0, 0),
        )
        nc.tensor.matmul(
            s_ps[:, 1, :],
            kT[(pair, jt)][64:128, jj * 128 : (jj + 1) * 128],
            qT[(it, pair)][64:128, :],
            start=True, stop=True,
            tile_position=(64, 0),
        )
        ex = ex_pool.tile([128, 2, IT], BF16, name=f"ex_{it}_{pair}_{jc}",
                          tag="ex")
        nc.scalar.activation(
            out=ex[:], in_=s_ps[:],
            func=mybir.ActivationFunctionType.Exp,
            scale=0.125,
        )
        nc.tensor.matmul(
            pc_a[0:65, :],
            v_t[jt][:, jj, ha * 65 : (ha + 1) * 65],
            ex[:, 0, :],
            start=(jc == 0), stop=(jc == JC - 1),
        )
        nc.tensor.matmul(
            pc_b[0:65, :],
            v_t[jt][:, jj, hb * 65 : (hb + 1) * 65],
            ex[:, 1, :],
            start=(jc == 0), stop=(jc == JC - 1),
        )

    def _normalize(it, pair, pc_a, pc_b):
        """Evacuate [ctx; denom] PSUM, reciprocal + PE-broadcast, scale."""
        isl = slice(it * IT, (it + 1) * IT)
        ha, hb = 2 * pair, 2 * pair + 1
        for head, pc in ((ha, pc_a), (hb, pc_b)):
            ce = cep.tile([128, IT], F32, name=f"ce_{it}_{head}", tag="ce")
            nc.vector.tensor_copy(ce[0:65, :], pc[0:65, :])
            rc = small.tile([128, IT], BF16, name=f"rc_{it}_{head}", tag="rc")
            with nc.allow_low_precision(
                reason="bf16 softmax-denominator reciprocal feeds the PE "
                       "broadcast matmul; 0.4% relative error is ~1e-4 of "
                       "the final output"
            ):
                nc.vector.reciprocal(rc[64:65, :], ce[64:65, :])
            bc_ps = ps_proj.tile([128, IT], F32, name=f"bc_{it}_{head}",
                                 tag="pp")
            nc.tensor.matmul(
                bc_ps[0:64, :],
                ones_t[64:65, :],
                rc[64:65, :],
                start=True, stop=True,
                tile_position=(64, 0),
            )
            chunk = head // 2
            if head % 2 == 0:
                nc.vector.tensor_tensor(
                    out=ctxT_sb[0:64, chunk, isl],
                    in0=ce[0:64, :], in1=bc_ps[0:64, :],
                    op=mybir.AluOpType.mult,
                )
            else:
                sc = small.tile([64, IT], BF16, name=f"sc_{it}_{head}",
                                tag="scm")
                nc.vector.tensor_tensor(
                    out=sc[:], in0=ce[0:64, :], in1=bc_ps[0:64, :],
                    op=mybir.AluOpType.mult,
                )
                nc.gpsimd.dma_start(
                    out=ctxT_sb[64:128, chunk, isl], in_=sc[:],
                )

    def _attn_pair(it, pair):
        pc_a = ps_ctx.tile([128, IT], F32, name=f"pca_{it}_{pair}", tag="pc")
        pc_b = ps_ctx.tile([128, IT], F32, name=f"pcb_{it}_{pair}", tag="pc")
        for jc in range(JC):
            _attn_jc(it, pair, jc, pc_a, pc_b)
        _normalize(it, pair, pc_a, pc_b)

    def _outproj_band(it, do_tail, split_evac=False):
        for i2 in range(4):
            ic = it * 4 + i2
            ot = evac.tile([128, E], BF16, name=f"ot_{ic}", tag="ot")
            for et in range(2):
                po = ps_proj.tile([128, IT], F32, name=f"po_{ic}_{et}",
                                  tag="pp")
                for hc in range(2):
                    nc.tensor.matmul(
                        po[:],
                        ctxT_sb[:, hc, ic * 128 : (ic + 1) * 128],
                        w_sb["wo"][:, hc, et * 512 : (et + 1) * 512],
                        start=(hc == 0),
                        stop=(hc == 1),
                    )
                dst = ot[:, et * 512 : (et + 1) * 512]
                if split_evac and et == 0:
                    nc.scalar.copy(dst, po[:])
                else:
                    nc.vector.tensor_copy(dst, po[:])
            nc.sync.dma_start(
                out=rs_in[it][i2 * 128 : (i2 + 1) * 128, :], in_=ot[:]
            )
        if do_tail:
            nc.gpsimd.collective_compute(
                "ReduceScatter",
                mybir.AluOpType.add,
                replica_groups=GROUPS,
                ins=[rs_in[it].ap()],
                outs=[rs_out[it].ap()],
            )

    def _ln_tail(do_tail, xr_tiles=None):
        for band in (range(N_IT) if do_tail else []):
            xb = lnp.tile([128, E], BF16, name=f"lb_{band}", tag="lb")
            nc.sync.dma_start(out=xb[:], in_=rs_out[band].ap())
            xr = xr_tiles[band]
            xt = lnp.tile([128, E], F32, name=f"lx_{band}", tag="lx")
            nc.vector.tensor_tensor(out=xt[:], in0=xb[:], in1=xr[:],
                                    op=mybir.AluOpType.add)
            nc.vector.tensor_tensor(out=xt[:], in0=xt[:], in1=bo_bc[:],
                                    op=mybir.AluOpType.add)
            stats = small.tile([128, 2, 6], F32, name=f"st_{band}", tag="st")
            for h in range(2):
                nc.vector.bn_stats(out=stats[:, h, :],
                                   in_=xt[:, h * 512 : (h + 1) * 512])
            mv = small.tile([128, 2], F32, name=f"mv_{band}", tag="mv")
            nc.vector.bn_aggr(out=mv[:], in_=stats.rearrange("p a b -> p (a b)"))
            rstd = small.tile([128, 1], F32, name=f"rstd_{band}", tag="rstd")
            nc.scalar.activation(
                out=rstd[:], in_=mv[:, 1:2],
                func=mybir.ActivationFunctionType.Sqrt,
                bias=eps_sb[:],
            )
            nc.vector.reciprocal(rstd[:], rstd[:])
            nc.vector.tensor_scalar(
                out=xt[:], in0=xt[:],
                scalar1=mv[:, 0:1], scalar2=rstd[:],
                op0=mybir.AluOpType.subtract,
                op1=mybir.AluOpType.mult,
            )
            nc.vector.tensor_tensor(out=xt[:], in0=xt[:], in1=gamma_bc[:],
                                    op=mybir.AluOpType.mult)
            nc.vector.tensor_tensor(out=xt[:], in0=xt[:], in1=beta_bc[:],
                                    op=mybir.AluOpType.add)
            nc.sync.dma_start(out=out[band], in_=xt[:])

    def body(do_tail=True):
        # input DMAs in priority order: q tile 0, then the four kv tiles,
        # then the remaining q tiles
        xq_tiles = []
        for it in range(N_IT):
            xt = xtp.tile([128, EC, IT], BF16, name=f"xqT_{it}", tag=f"xq{it}")
            xq_tiles.append(xt)
        nc.sync.dma_start(
            out=xq_tiles[0][:],
            in_=xqT.rearrange("(c p) n -> p c n", p=128)[:, :, 0:IT],
        )
        kv_tiles = []
        for jt in range(N_IT):
            xt = kvp.tile([128, EC, IT], BF16, name=f"xkvT_{jt}", tag=f"xkv{jt}")
            nc.sync.dma_start(
                out=xt[:],
                in_=xkvT.rearrange("(c p) n -> p c n", p=128)[
                    :, :, jt * IT : (jt + 1) * IT
                ],
            )
            kv_tiles.append(xt)
        for it in range(1, N_IT):
            nc.sync.dma_start(
                out=xq_tiles[it][:],
                in_=xqT.rearrange("(c p) n -> p c n", p=128)[
                    :, :, it * IT : (it + 1) * IT
                ],
            )

        # residual-row prefetch for the LayerNorm tail
        xr_tiles = []
        for band in range(N_IT):
            xr = singles.tile([128, E], F32, name=f"lr_{band}", tag=f"lr{band}")
            nc.sync.dma_start(out=xr[:], in_=xqr[band])
            xr_tiles.append(xr)

        # q projection for (band 0, pair 0), then kv projections fused with
        # attention for (it=0, pair=0). Work not needed for the fused stream
        # (pair-1 k/v/q projections, early bands' output projections) is
        # deprioritized so the Tile scheduler uses it to fill PE slack under
        # the ACT-bound exp stream instead of preempting score matmuls.
        DEFER = 10**6
        _proj_q(0, xq_tiles[0], pairs=(0,))
        pc00_a = ps_ctx.tile([128, IT], F32, name="pca_0_0", tag="pc")
        pc00_b = ps_ctx.tile([128, IT], F32, name="pcb_0_0", tag="pc")
        for jt in range(N_IT):
            _proj_k(jt, 0, kv_tiles[jt])
            _proj_v(jt, kv_tiles[jt], 0)
            for jj in range(4):
                _attn_jc(0, 0, jt * 4 + jj, pc00_a, pc00_b)
            with tc.high_priority(offset=-DEFER):
                _proj_k(jt, 1, kv_tiles[jt])
                _proj_v(jt, kv_tiles[jt], 1)
        with tc.high_priority(offset=-DEFER):
            _proj_q(0, xq_tiles[0], pairs=(1,))
        _normalize(0, 0, pc00_a, pc00_b)
        with tc.high_priority(offset=-DEFER):
            _proj_q(1, xq_tiles[1])
        _attn_pair(0, 1)
        for it in range(1, N_IT):
            _attn_pair(it, 0)
            if it < N_IT - 1:
                with tc.high_priority(offset=-DEFER):
                    _proj_q(it + 1, xq_tiles[it + 1])
            with tc.high_priority(offset=-DEFER):
                _outproj_band(it - 1, do_tail)
            _attn_pair(it, 1)
        _outproj_band(N_IT - 1, do_tail, split_evac=True)
        _ln_tail(do_tail, xr_tiles)

    if iters == 1:
        body()
    else:
        with tc.For_i(0, iters):
            body(do_tail=False)
        body()

    ctx.close()


def _prepare_inputs(query_seq, key_value_seq, Wq, bq, Wk, bk, Wv, bv, Wo, bo,
                    ln_gamma, ln_beta):
    """Build the 8 per-core input tuples (host-side transpose + bf16 cast)."""
    ins = []
    xT = [np.ascontiguousarray(query_seq[b].T).astype(NP_BF16) for b in range(2)]
    kvT = [np.ascontiguousarray(key_value_seq[b].T).astype(NP_BF16)
           for b in range(2)]
    for c in range(8):
        b, r = divmod(c, 4)
        hs = slice(256 * r, 256 * (r + 1))
        wqT = np.ascontiguousarray(Wq[hs, :].T).astype(NP_BF16)
        wkT = np.ascontiguousarray(Wk[hs, :].T).astype(NP_BF16)
        wvT = np.ascontiguousarray(Wv[hs, :].T).astype(NP_BF16)
        woT = np.ascontiguousarray(Wo[:, hs].T).astype(NP_BF16)
        bqk = np.stack(
            [bq[hs][:128], bq[hs][128:], bk[hs][:128], bk[hs][128:]], axis=1
        ).astype(np.float32)
        bvs = np.ascontiguousarray(bv[hs])
        # residual rows: band k covers batch rows [512k + 128r, 512k + 128(r+1))
        xqr = np.stack(
            [query_seq[b, 512 * k + 128 * r : 512 * k + 128 * (r + 1)]
             for k in range(4)]
        )
        ins.append((xT[b], kvT[b], wqT, wkT, wvT, woT, bqk, bvs,
                    np.ascontiguousarray(bo), np.ascontiguousarray(ln_gamma),
                    np.ascontiguousarray(ln_beta), xqr))
    return ins


def kernel(**inputs) -> np.ndarray:
    query_seq = np.asarray(inputs["query_seq"], dtype=np.float32)
    key_value_seq = np.asarray(inputs["key_value_seq"], dtype=np.float32)
    args = {
        k: np.asarray(inputs[k], dtype=np.float32)
        for k in ("Wq", "bq", "Wk", "bk", "Wv", "bv", "Wo", "bo",
                  "ln_gamma", "ln_beta")
    }
    ins = _prepare_inputs(query_seq, key_value_seq, **args)
    out_like = [(np.zeros((4, 128, E), np.float32),) for _ in range(8)]
    res = run_kernel(
        make_attention_kernel(1),
        None,
        ins,
        bass_type=tile.TileContext,
        num_cores=8,
        check_with_sim=False,
        check_with_hw=True,
        trace_hw=False,
        output_like=out_like,
    )
    out = np.empty((B, L, E), np.float32)
    for c in range(8):
        bnd = res.results[c]["0_dram"]  # [4, 128, 1024]
        b, r = divmod(c, 4)
        for k in range(4):
            out[b, 512 * k + 128 * r : 512 * k + 128 * (r + 1), :] = bnd[k]
    return out
